# revision 20
# baseline (speedup 1.0000x reference)
"""Trainium2 Bass kernel for nn_Augmenter_separate (dense_mlp).

Computation (reference):
    zc = einsum('kbd,dh->kbh', z, W1[:64])          # (4, 64, 128)
    ic = items @ W1[64:]                             # (8192, 128)
    for k in 4:
        h1 = relu(zc[k][:,None,:] + ic[None] + b1)   # (64, 8192, 128)
        h2 = relu(h1 @ W2 + b2)
        w  = ((h2 @ W3)[...,0] + b3) * cates[:,k]
        p  = sigmoid((log(delta[k]) - log1p(-delta[k]) + w) / 0.2)
        reg += sum(p)/p.size
        prob_k = where(x==1, 1-p, p)
    out = mean_k(prob_k), reg/3

Sharding: data-parallel over batch. Each of the 8 cores processes 8 of the
64 batch rows (full item axis). The scalar reg is summed on the host from
per-core partials; the (64, 8192) output is gathered by concatenation.

Per-core dataflow (all matmuls in float32r, fp32 storage):
  - icT [h=128, n=8192] once:  W1b^T @ itemsT          (PE)
  - zcb [h=128, kb=32]  once:  W1a^T @ zT + b1         (PE+ACT)
  - per (k,b) pair kb and 2048-wide n-chunk t4:
      h1 [128,2048] = relu(icT + zcb[:,kb])            (DVE tensor_scalar)
      psum = W2^T @ h1                                 (PE, 4x 512)
      h2 = relu(psum + b2)                             (ACT / DVE split)
      wt[32j+kb, :] += W3row-mask @ h2[:, j*512:...]   (PE, col-tiled)
  - epilogue per t4 on [128,512] packed layout (row = 32j + kb):
      u = wt * cates + L0  ;  p = sigmoid(5u)  (reg via accum_out)
      psum_f[b,:] = sum_k p  (PE row-tiled selection matmul)
      out = xf + 0.25*(1-2x)*psum_f                    (DVE)
"""

import numpy as np

import concourse.bass as bass
import concourse.bacc as bacc
import concourse.tile as tile
import concourse.mybir as mybir
from concourse.bass import AP  # noqa: F401

F32 = mybir.dt.float32
F32R = mybir.dt.float32r
I32 = mybir.dt.int32
AF = mybir.ActivationFunctionType
OP = mybir.AluOpType

KFAC = 4
DFAC = 64
HDIM = 128
NITEMS = 8192
BATCH = 64
NCORES = 8
BLOC = BATCH // NCORES          # 8 local batch rows
KB = KFAC * BLOC                # 32 (k,b) pairs per core
NT4 = 4                         # n-chunks of 2048
T4 = NITEMS // NT4              # 2048
TM = 512                        # matmul moving-dim tile
TAU = 0.2

# kb iterations whose h2 evacuation runs on DVE instead of ACT,
# and kbs whose h1 runs on GPSIMD, interleaved to keep engines fed.
DVE_EVAC = frozenset({10, 21, 31})
GPS_H1 = frozenset({1, 5, 9, 13, 17, 21, 25, 29})


def _r(ap):
    return ap.bitcast(F32R)


def build_program():
    nc = bacc.Bacc("TRN2", target_bir_lowering=False, debug=False)

    # ---- DRAM I/O ----
    w1a_d = nc.dram_tensor("w1a", [DFAC, HDIM], F32R, kind="ExternalInput").ap()
    w1b_d = nc.dram_tensor("w1b", [DFAC, HDIM], F32R, kind="ExternalInput").ap()
    w2_d = nc.dram_tensor("w2", [HDIM, HDIM], F32R, kind="ExternalInput").ap()
    w3_d = nc.dram_tensor("w3", [HDIM, 1], F32, kind="ExternalInput").ap()
    b1_d = nc.dram_tensor("b1", [HDIM, 1], F32, kind="ExternalInput").ap()
    b2_d = nc.dram_tensor("b2", [HDIM, 1], F32, kind="ExternalInput").ap()
    b3_d = nc.dram_tensor("b3b", [HDIM, 1], F32, kind="ExternalInput").ap()
    itemsT_d = nc.dram_tensor("itemsT", [DFAC, NITEMS], F32R, kind="ExternalInput").ap()
    zt_d = nc.dram_tensor("zt", [DFAC, KB], F32R, kind="ExternalInput").ap()
    delta_d = nc.dram_tensor("delta_l", [128, T4], F32, kind="ExternalInput").ap()
    cates_d = nc.dram_tensor("cates_p", [128, T4], F32, kind="ExternalInput").ap()
    x_d = nc.dram_tensor("x_l", [BLOC, NITEMS], I32, kind="ExternalInput").ap()
    sel_d = nc.dram_tensor("sel", [128, BLOC], F32R, kind="ExternalInput").ap()

    out_d = nc.dram_tensor("out_p", [BLOC, NITEMS], F32, kind="ExternalOutput").ap()
    rs_d = nc.dram_tensor("rs", [128, NT4], F32, kind="ExternalOutput").ap()

    with tile.TileContext(nc) as tc:
        with tc.tile_pool(name="const", bufs=1) as const:
            # small constants
            w1a = const.tile([DFAC, HDIM], F32R)
            w1b = const.tile([DFAC, HDIM], F32R)
            w2r = const.tile([HDIM, HDIM], F32R)
            w3 = const.tile([HDIM, 1], F32)
            b1 = const.tile([HDIM, 1], F32)
            b2 = const.tile([HDIM, 1], F32)
            b3 = const.tile([HDIM, 1], F32)
            zt = const.tile([DFAC, KB], F32R)
            selr = const.tile([128, BLOC], F32R)
            for dst, src in (
                (w1a, w1a_d), (w1b, w1b_d), (w2r, w2_d), (w3, w3_d),
                (b1, b1_d), (b2, b2_d), (b3, b3_d), (zt, zt_d), (selr, sel_d),
            ):
                nc.sync.dma_start(out=dst, in_=src)

            # big persistent tensors
            zcb = const.tile([HDIM, KB], F32)
            l0p = const.tile([128, T4], F32)
            cates = const.tile([128, T4], F32)
            m1q = const.tile([BLOC, NITEMS], F32)
            xf = const.tile([BLOC, NITEMS], F32)
            # one-hot masks: masks3[:, kb, :] is [128,32] with col kb = W3
            masks3 = const.tile([HDIM, KB, KB], F32R)
            icTr = const.tile([HDIM, NITEMS], F32R)
            rs = const.tile([128, NT4], F32)

            # ---- setup ----
            from contextlib import ExitStack
            with ExitStack() as setup_ctx:
                spool = setup_ctx.enter_context(
                    tc.tile_pool(name="setup_sb", bufs=1))
                spsum = setup_ctx.enter_context(
                    tc.tile_pool(name="setup_ps", bufs=2, space="PSUM"))

                itemsT = spool.tile([DFAC, NITEMS], F32R, tag="itemsT")
                for dc in range(4):
                    cs = slice(dc * (NITEMS // 4), (dc + 1) * (NITEMS // 4))
                    nc.sync.dma_start(out=itemsT[:, cs], in_=itemsT_d[:, cs])

                # icT = W1b^T @ itemsT
                for it in range(NITEMS // TM):
                    ps = spsum.tile([HDIM, TM], F32, tag="ps")
                    nc.tensor.matmul(
                        ps, w1b, itemsT[:, it * TM:(it + 1) * TM],
                        start=True, stop=True)
                    nc.scalar.activation(
                        out=icTr[:, it * TM:(it + 1) * TM], in_=ps,
                        func=AF.Copy)

                # zcb = W1a^T @ zT + b1
                psz = spsum.tile([HDIM, KB], F32, tag="ps")
                nc.tensor.matmul(psz, w1a, zt, start=True, stop=True)
                nc.scalar.activation(out=zcb, in_=psz, func=AF.Identity, bias=b1)

                # L0 = log(delta) - log(1-delta)  (+ b3*cates), packed layout
                delta = spool.tile([128, T4], F32, tag="delta")
                nc.sync.dma_start(out=delta, in_=delta_d)
                tln = spool.tile([128, T4], F32, tag="tln")
                nc.scalar.activation(out=tln, in_=delta, func=AF.Ln)
                tln2 = spool.tile([128, T4], F32, tag="tln2")
                nc.scalar.activation(
                    out=tln2, in_=delta, func=AF.Ln, scale=-1.0, bias=1.0)
                nc.gpsimd.tensor_tensor(
                    out=l0p, in0=tln, in1=tln2, op=OP.subtract)

                nc.sync.dma_start(out=cates, in_=cates_d)
                tb3 = spool.tile([128, T4], F32, tag="tln")
                nc.gpsimd.tensor_scalar(
                    out=tb3, in0=cates, scalar1=b3, scalar2=None, op0=OP.mult)
                nc.gpsimd.tensor_tensor(out=l0p, in0=l0p, in1=tb3, op=OP.add)

                # x-derived tensors: m1q = 0.25*(1-2x), xf = float(x)
                xi = spool.tile([BLOC, NITEMS], I32, tag="xi")
                nc.sync.dma_start(out=xi, in_=x_d)
                nc.gpsimd.tensor_scalar(
                    out=m1q, in0=xi, scalar1=-0.5, scalar2=0.25,
                    op0=OP.mult, op1=OP.add)
                nc.gpsimd.tensor_copy(out=xf, in_=xi)

                # one-hot masks for the W3-row contraction
                # (memset can't produce f32r; zero via fp32 temp + copy)
                mz = spool.tile([HDIM, KB * KB], F32, tag="tln")
                nc.gpsimd.memset(mz, 0.0)
                nc.gpsimd.tensor_copy(
                    out=masks3.rearrange("p a b -> p (a b)"), in_=mz)
                for kb in range(KB):
                    nc.gpsimd.tensor_copy(
                        out=masks3[:, kb, kb:kb + 1], in_=w3)

            # ---- main loop ----
            with (
                tc.tile_pool(name="h1p", bufs=3) as h1p,
                tc.tile_pool(name="h2p", bufs=6) as h2p,
                tc.tile_pool(name="hps", bufs=2, space="PSUM") as hps,
                tc.tile_pool(name="wps", bufs=4, space="PSUM") as wps,
                tc.tile_pool(name="epi", bufs=2) as epi,
                tc.tile_pool(name="outp", bufs=2) as outp,
            ):
                for t4 in range(NT4):
                    n0 = t4 * T4
                    wtl = [wps.tile([KB, TM], F32, tag="w",
                                    name=f"wt_{t4}_{j}")
                           for j in range(NT4)]
                    for kb in range(KB):
                        h1 = h1p.tile([HDIM, T4], F32R, tag="h1")
                        h1_eng = nc.gpsimd if kb in GPS_H1 else nc.vector
                        h1_eng.tensor_scalar(
                            out=h1, in0=icTr[:, n0:n0 + T4],
                            scalar1=zcb[:, kb:kb + 1], scalar2=0.0,
                            op0=OP.add, op1=OP.max)
                        h2h = [h2p.tile([HDIM, 2 * TM], F32R, tag="h2",
                                        name=f"h2_{t4}_{kb}_{hf}")
                               for hf in range(2)]
                        for half in range(2):
                            ph = hps.tile([HDIM, 2 * TM], F32, tag="ph")
                            for q in range(2):
                                c = half * 2 * TM + q * TM
                                nc.tensor.matmul(
                                    ph[:, q * TM:(q + 1) * TM], w2r,
                                    h1[:, c:c + TM], start=True, stop=True)
                            hs = h2h[half]
                            if kb not in DVE_EVAC:
                                nc.scalar.activation(
                                    out=hs, in_=ph, func=AF.Relu, bias=b2)
                            else:
                                nc.vector.tensor_scalar(
                                    out=hs, in0=ph, scalar1=b2, scalar2=0.0,
                                    op0=OP.add, op1=OP.max)
                            for q in range(2):
                                j = half * 2 + q
                                nc.tensor.matmul(
                                    wtl[j],
                                    masks3[:, kb, :],
                                    hs[:, q * TM:(q + 1) * TM],
                                    start=(kb == 0), stop=(kb == KB - 1))

                    # epilogue for this n-chunk
                    ub = epi.tile([128, TM], F32, tag="ub")
                    for j in range(NT4):
                        nc.vector.tensor_tensor(
                            out=ub[32 * j:32 * (j + 1), :], in0=wtl[j],
                            in1=cates[32 * j:32 * (j + 1),
                                      t4 * TM:(t4 + 1) * TM],
                            op=OP.mult)
                    nc.vector.tensor_tensor(
                        out=ub, in0=ub, in1=l0p[:, t4 * TM:(t4 + 1) * TM],
                        op=OP.add)
                    p = epi.tile([128, TM], F32R, tag="p")
                    nc.scalar.activation(
                        out=p, in_=ub, func=AF.Sigmoid, scale=1.0 / TAU,
                        accum_out=rs[:, t4:t4 + 1])
                    ost = outp.tile([BLOC, T4], F32, tag="ost")
                    for j in range(NT4):
                        pf = wps.tile([BLOC, TM], F32, tag="w")
                        nc.tensor.matmul(
                            pf, selr[32 * j:32 * (j + 1), :],
                            p[32 * j:32 * (j + 1), :],
                            start=True, stop=True, tile_position=(32 * j, 0))
                        js = slice(j * TM, (j + 1) * TM)
                        ns = slice(n0 + j * TM, n0 + (j + 1) * TM)
                        nc.vector.tensor_tensor(
                            out=ost[:, js], in0=pf, in1=m1q[:, ns],
                            op=OP.mult)
                        nc.vector.tensor_tensor(
                            out=ost[:, js], in0=ost[:, js], in1=xf[:, ns],
                            op=OP.add)
                    nc.sync.dma_start(out=out_d[:, n0:n0 + T4], in_=ost)
                nc.sync.dma_start(out=rs_d, in_=rs)

    nc.compile()
    return nc


def make_in_maps(cates, h, z, x, delta, items, W1, b1, W2, b2, W3, b3):
    """Host-side sharding + layout prep (pure slicing/transpose/broadcast)."""
    cates = np.asarray(cates, np.float32)
    z = np.asarray(z, np.float32)
    x = np.ascontiguousarray(np.asarray(x, np.int32))
    delta = np.asarray(delta, np.float32)
    items = np.asarray(items, np.float32)
    W1 = np.asarray(W1, np.float32)
    W2 = np.ascontiguousarray(np.asarray(W2, np.float32))
    W3 = np.ascontiguousarray(np.asarray(W3, np.float32).reshape(HDIM, 1))
    b1 = np.ascontiguousarray(np.asarray(b1, np.float32).reshape(HDIM, 1))
    b2 = np.ascontiguousarray(np.asarray(b2, np.float32).reshape(HDIM, 1))
    b3b = np.full((HDIM, 1), np.float32(np.asarray(b3).reshape(-1)[0]),
                  np.float32)

    itemsT = np.ascontiguousarray(items.T)                      # (64, 8192)
    w1a = np.ascontiguousarray(W1[:DFAC])
    w1b = np.ascontiguousarray(W1[DFAC:])
    sel = np.ascontiguousarray(
        np.tile(np.eye(BLOC, dtype=np.float32), (16, 1)))       # (128, 8)

    # packed epilogue layout: row = 32*j + (k*8+b_local), col = t4*512 + c
    # with n = t4*2048 + j*512 + c
    def pack128(a32):  # (32, 8192) -> (128, 2048)
        return np.ascontiguousarray(
            a32.reshape(KB, NT4, NT4, TM).transpose(2, 0, 1, 3)
            .reshape(128, T4))

    catesT = cates.T                                            # (4, 8192)
    cates_rep = np.broadcast_to(
        catesT[:, None, :], (KFAC, BLOC, NITEMS)).reshape(KB, NITEMS)
    cates_p = pack128(cates_rep)

    in_maps = []
    for c in range(NCORES):
        bs = slice(c * BLOC, (c + 1) * BLOC)
        zt = np.ascontiguousarray(
            z[:, bs, :].transpose(2, 0, 1).reshape(DFAC, KB))   # (64, 32)
        delta_l = pack128(delta[:, bs, :].reshape(KB, NITEMS))
        x_l = np.ascontiguousarray(x[bs])
        in_maps.append({
            "w1a": w1a, "w1b": w1b, "w2": W2, "w3": W3,
            "b1": b1, "b2": b2, "b3b": b3b,
            "itemsT": itemsT, "zt": zt,
            "delta_l": delta_l, "cates_p": cates_p, "x_l": x_l,
            "sel": sel,
        })
    return in_maps


_NC_CACHE = None


def _get_program():
    global _NC_CACHE
    if _NC_CACHE is None:
        _NC_CACHE = build_program()
    return _NC_CACHE


def kernel(**inputs):
    from concourse.bass_utils import run_bass_kernel_spmd
    nc = _get_program()
    in_maps = make_in_maps(**inputs)
    res = run_bass_kernel_spmd(nc, in_maps, core_ids=list(range(NCORES)))
    outs = res.results
    all_item_prob = np.concatenate(
        [outs[c]["out_p"] for c in range(NCORES)], axis=0)
    reg_total = np.sum([outs[c]["rs"].sum(dtype=np.float64)
                        for c in range(NCORES)])
    reg_loss_aug = np.float32(reg_total / (BATCH * NITEMS) / (KFAC - 1))
    return all_item_prob, np.asarray(reg_loss_aug, dtype=np.float32)


# revision 36
# speedup vs baseline: 4.0709x; 4.0709x over previous
"""Trainium2 Bass kernel for nn_Augmenter_separate (dense_mlp).

Computation (reference):
    zc = einsum('kbd,dh->kbh', z, W1[:64])          # (4, 64, 128)
    ic = items @ W1[64:]                             # (8192, 128)
    for k in 4:
        h1 = relu(zc[k][:,None,:] + ic[None] + b1)   # (64, 8192, 128)
        h2 = relu(h1 @ W2 + b2)
        w  = ((h2 @ W3)[...,0] + b3) * cates[:,k]
        p  = sigmoid((log(delta[k]) - log1p(-delta[k]) + w) / 0.2)
        reg += sum(p)/p.size
        prob_k = where(x==1, 1-p, p)
    out = mean_k(prob_k), reg/3

Sharding: data-parallel over batch. Each of the 8 cores processes 8 of the
64 batch rows (full item axis). The scalar reg is summed on the host from
per-core partials; the (64, 8192) output is gathered by concatenation.

Per-core dataflow (all matmuls in float32r, fp32 storage):
  - icT [h=128, n=8192] once:  W1b^T @ itemsT          (PE)
  - zcb [h=128, kb=32]  once:  W1a^T @ zT + b1         (PE+ACT)
  - per (k,b) pair kb and 2048-wide n-chunk t4:
      h1 [128,2048] = relu(icT + zcb[:,kb])            (DVE tensor_scalar)
      psum = W2^T @ h1                                 (PE, 4x 512)
      h2 = relu(psum + b2)                             (ACT / DVE split)
      wt[32j+kb, :] += W3row-mask @ h2[:, j*512:...]   (PE, one-hot mask)
  - epilogue per t4 on [128,512] packed layout (row = 32j + kb):
      u = wt * cates + L0  ;  p = sigmoid(5u)  (reg via accum_out)
      psum_f[b,:] = sum_k p  (PE row-tiled selection matmul)
      out = xf + 0.25*(1-2x)*psum_f                    (DVE)
"""

import numpy as np

import concourse.bacc as bacc
import concourse.tile as tile
import concourse.mybir as mybir

F32 = mybir.dt.float32
F32R = mybir.dt.float32r
I32 = mybir.dt.int32
AF = mybir.ActivationFunctionType
OP = mybir.AluOpType

KFAC = 4
DFAC = 64
HDIM = 128
NITEMS = 8192
BATCH = 64
NCORES = 8
BLOC = BATCH // NCORES          # 8 local batch rows
KB = KFAC * BLOC                # 32 (k,b) pairs per core
NT4 = 4                         # n-chunks of 2048
T4 = NITEMS // NT4              # 2048
TM = 512                        # matmul moving-dim tile
TAU = 0.2

# kb iterations whose h2 evacuation runs on DVE instead of ACT,
# and kbs whose h1 runs on GPSIMD, interleaved to keep engines fed.
DVE_EVAC = frozenset({4, 10, 16, 22, 28})


def build_program():
    nc = bacc.Bacc("TRN2", target_bir_lowering=False, debug=False)

    # ---- DRAM I/O ----
    w1a_d = nc.dram_tensor("w1a", [DFAC, HDIM], F32R, kind="ExternalInput").ap()
    w1b_d = nc.dram_tensor("w1b", [DFAC, HDIM], F32R, kind="ExternalInput").ap()
    w2_d = nc.dram_tensor("w2", [HDIM, HDIM], F32R, kind="ExternalInput").ap()
    w3_d = nc.dram_tensor("w3", [HDIM, 1], F32, kind="ExternalInput").ap()
    b1_d = nc.dram_tensor("b1", [HDIM, 1], F32, kind="ExternalInput").ap()
    b2_d = nc.dram_tensor("b2", [HDIM, 1], F32, kind="ExternalInput").ap()
    b3_d = nc.dram_tensor("b3b", [HDIM, 1], F32, kind="ExternalInput").ap()
    itemsT_d = nc.dram_tensor("itemsT", [DFAC, NITEMS], F32R, kind="ExternalInput").ap()
    zt_d = nc.dram_tensor("zt", [DFAC, KB], F32R, kind="ExternalInput").ap()
    delta_d = nc.dram_tensor("delta_l", [128, T4], F32, kind="ExternalInput").ap()
    cates_d = nc.dram_tensor("cates_p", [128, T4], F32, kind="ExternalInput").ap()
    x_d = nc.dram_tensor("x_l", [BLOC, NITEMS], I32, kind="ExternalInput").ap()
    sel_d = nc.dram_tensor("sel", [128, BLOC], F32R, kind="ExternalInput").ap()

    out_d = nc.dram_tensor("out_p", [BLOC, NITEMS], F32, kind="ExternalOutput").ap()
    rs_d = nc.dram_tensor("rs", [128, NT4], F32, kind="ExternalOutput").ap()

    with tile.TileContext(nc) as tc:
        with tc.tile_pool(name="const", bufs=1) as const:
            # small constants
            w1a = const.tile([DFAC, HDIM], F32R)
            w1b = const.tile([DFAC, HDIM], F32R)
            w2r = const.tile([HDIM, HDIM], F32R)
            w3 = const.tile([HDIM, 1], F32)
            b1 = const.tile([HDIM, 1], F32)
            b2 = const.tile([HDIM, 1], F32)
            b3 = const.tile([HDIM, 1], F32)
            zt = const.tile([DFAC, KB], F32R)
            selr = const.tile([128, BLOC], F32R)
            for dst, src in (
                (w1a, w1a_d), (w1b, w1b_d), (w2r, w2_d), (w3, w3_d),
                (b1, b1_d), (b2, b2_d), (b3, b3_d), (zt, zt_d), (selr, sel_d),
            ):
                nc.sync.dma_start(out=dst, in_=src)

            # big persistent tensors
            zcb = const.tile([HDIM, KB], F32)
            l0p = const.tile([128, T4], F32)
            cates = const.tile([128, T4], F32)
            m1q = const.tile([BLOC, NITEMS], F32)
            xf = const.tile([BLOC, NITEMS], F32)
            # sliding-window one-hot mask: col 127 = W3, zeros elsewhere.
            # mwin[:, 127-m : 255-m] is a [128,128] matrix with col m = W3.
            mwin = const.tile([HDIM, 255], F32R)
            icTr = const.tile([HDIM, NITEMS], F32R)
            rs = const.tile([128, NT4], F32)

            # ---- setup ----
            from contextlib import ExitStack
            with ExitStack() as setup_ctx:
                spool = setup_ctx.enter_context(
                    tc.tile_pool(name="setup_sb", bufs=1))
                spsum = setup_ctx.enter_context(
                    tc.tile_pool(name="setup_ps", bufs=2, space="PSUM"))

                # icT = W1b^T @ itemsT
                for it in range(NITEMS // TM):
                    ps = spsum.tile([HDIM, TM], F32, tag="ps")
                    nc.tensor.matmul(
                        ps, w1b, itemsT[:, it * TM:(it + 1) * TM],
                        start=True, stop=True)
                    nc.scalar.activation(
                        out=icTr[:, it * TM:(it + 1) * TM], in_=ps,
                        func=AF.Copy)

                # zcb = W1a^T @ zT + b1
                psz = spsum.tile([HDIM, KB], F32, tag="ps")
                nc.tensor.matmul(psz, w1a, zt, start=True, stop=True)
                nc.scalar.activation(out=zcb, in_=psz, func=AF.Identity, bias=b1)

                # L0 = log(delta) - log(1-delta)  (+ b3*cates), packed layout
                delta = spool.tile([128, T4], F32, tag="delta")
                nc.sync.dma_start(out=delta, in_=delta_d)
                tln = spool.tile([128, T4], F32, tag="tln")
                nc.scalar.activation(out=tln, in_=delta, func=AF.Ln)
                tln2 = spool.tile([128, T4], F32, tag="tln2")
                nc.scalar.activation(
                    out=tln2, in_=delta, func=AF.Ln, scale=-1.0, bias=1.0)
                nc.vector.tensor_tensor(
                    out=l0p, in0=tln, in1=tln2, op=OP.subtract)

                nc.sync.dma_start(out=cates, in_=cates_d)
                tb3 = spool.tile([128, T4], F32, tag="tln")
                nc.vector.tensor_scalar(
                    out=tb3, in0=cates, scalar1=b3, scalar2=None, op0=OP.mult)
                nc.vector.tensor_tensor(out=l0p, in0=l0p, in1=tb3, op=OP.add)

                # x-derived tensors: m1q = 0.25*(1-2x), xf = float(x)
                xi = spool.tile([BLOC, NITEMS], I32, tag="xi")
                nc.sync.dma_start(out=xi, in_=x_d)
                nc.gpsimd.tensor_scalar(
                    out=m1q, in0=xi, scalar1=-0.5, scalar2=0.25,
                    op0=OP.mult, op1=OP.add)
                nc.gpsimd.tensor_copy(out=xf, in_=xi)

                # one-hot masks for the W3-row contraction
                # (memset can't produce f32r; zero via fp32 temp + copy)
                mz = spool.tile([HDIM, KB * KB], F32, tag="tln")
                nc.vector.memset(mz, 0.0)
                nc.vector.tensor_copy(
                    out=masks3.rearrange("p a b -> p (a b)"), in_=mz)
                for kb in range(KB):
                    nc.vector.tensor_copy(
                        out=masks3[:, kb, kb:kb + 1], in_=w3)

            # ---- main loop ----
            with (
                tc.tile_pool(name="h1p", bufs=4) as h1p,
                tc.tile_pool(name="h2p", bufs=4) as h2p,
                tc.tile_pool(name="hps", bufs=3, space="PSUM") as hps,
                tc.tile_pool(name="wps", bufs=2, space="PSUM") as wps,
                tc.tile_pool(name="epi", bufs=2) as epi,
                tc.tile_pool(name="outp", bufs=2) as outp,
            ):
                h1_cache = {}
                pending_epi = None

                def make_h1(t4, kb):
                    h1 = h1p.tile([HDIM, T4], F32R, tag="h1",
                                  name=f"h1_{t4}_{kb}")
                    nc.vector.tensor_scalar(
                        out=h1, in0=icTr[:, t4 * T4:(t4 + 1) * T4],
                        scalar1=zcb[:, kb:kb + 1], scalar2=0.0,
                        op0=OP.add, op1=OP.max)
                    return h1

                for t4 in range(NT4):
                    n0 = t4 * T4
                    wt = wps.tile([128, TM], F32, tag="w",
                                  name=f"wt_{t4}")
                    for kb in range(KB):
                        h1 = h1p.tile([HDIM, T4], F32R, tag="h1")
                        nc.vector.tensor_scalar(
                            out=h1, in0=icTr[:, n0:n0 + T4],
                            scalar1=zcb[:, kb:kb + 1], scalar2=0.0,
                            op0=OP.add, op1=OP.max)
                        h2h = [h2p.tile([HDIM, 2 * TM], F32R, tag="h2",
                                        name=f"h2_{t4}_{kb}_{hf}")
                               for hf in range(2)]
                        for half in range(2):
                            ph = hps.tile([HDIM, 2 * TM], F32, tag="ph")
                            for q in range(2):
                                c = half * 2 * TM + q * TM
                                nc.tensor.matmul(
                                    ph[:, q * TM:(q + 1) * TM], w2r,
                                    h1[:, c:c + TM], start=True, stop=True)
                            hs = h2h[half]
                            if kb not in DVE_EVAC:
                                nc.scalar.activation(
                                    out=hs, in_=ph, func=AF.Relu, bias=b2)
                            else:
                                nc.vector.tensor_scalar(
                                    out=hs, in0=ph, scalar1=b2, scalar2=0.0,
                                    op0=OP.add, op1=OP.max)
                            for q in range(2):
                                j = half * 2 + q
                                nc.tensor.matmul(
                                    wtl[j],
                                    masks3[:, kb, :],
                                    hs[:, q * TM:(q + 1) * TM],
                                    start=(kb == 0), stop=(kb == KB - 1))

                    # epilogue for this n-chunk
                    ub = epi.tile([128, TM], F32, tag="ub")
                    for j in range(NT4):
                        nc.vector.tensor_tensor(
                            out=ub[32 * j:32 * (j + 1), :], in0=wtl[j],
                            in1=cates[32 * j:32 * (j + 1),
                                      t4 * TM:(t4 + 1) * TM],
                            op=OP.mult)
                    nc.vector.tensor_tensor(
                        out=ub, in0=ub, in1=l0p[:, t4 * TM:(t4 + 1) * TM],
                        op=OP.add)
                    p = epi.tile([128, TM], F32R, tag="p")
                    nc.scalar.activation(
                        out=p, in_=ub, func=AF.Sigmoid, scale=1.0 / TAU,
                        accum_out=rs[:, t4:t4 + 1])
                    ost = outp.tile([BLOC, T4], F32, tag="ost")
                    for j in range(NT4):
                        pf = wps.tile([BLOC, TM], F32, tag="w")
                        nc.tensor.matmul(
                            pf, selr[32 * j:32 * (j + 1), :],
                            p[32 * j:32 * (j + 1), :],
                            start=True, stop=True, tile_position=(32 * j, 0))
                        js = slice(j * TM, (j + 1) * TM)
                        ns = slice(n0 + j * TM, n0 + (j + 1) * TM)
                        nc.vector.tensor_tensor(
                            out=ost[:, js], in0=pf, in1=m1q[:, ns],
                            op=OP.mult)
                        nc.vector.tensor_tensor(
                            out=ost[:, js], in0=ost[:, js], in1=xf[:, ns],
                            op=OP.add)
                    nc.sync.dma_start(out=out_d[:, n0:n0 + T4], in_=ost)
                nc.sync.dma_start(out=rs_d, in_=rs)

    nc.compile()
    return nc


def make_in_maps(cates, h, z, x, delta, items, W1, b1, W2, b2, W3, b3):
    """Host-side sharding + layout prep (pure slicing/transpose/broadcast)."""
    cates = np.asarray(cates, np.float32)
    z = np.asarray(z, np.float32)
    x = np.ascontiguousarray(np.asarray(x, np.int32))
    delta = np.asarray(delta, np.float32)
    items = np.asarray(items, np.float32)
    W1 = np.asarray(W1, np.float32)
    W2 = np.ascontiguousarray(np.asarray(W2, np.float32))
    W3 = np.ascontiguousarray(np.asarray(W3, np.float32).reshape(HDIM, 1))
    b1 = np.ascontiguousarray(np.asarray(b1, np.float32).reshape(HDIM, 1))
    b2 = np.ascontiguousarray(np.asarray(b2, np.float32).reshape(HDIM, 1))
    b3b = np.full((HDIM, 1), np.float32(np.asarray(b3).reshape(-1)[0]),
                  np.float32)

    itemsT = np.ascontiguousarray(items.T)                      # (64, 8192)
    w1a = np.ascontiguousarray(W1[:DFAC])
    w1b = np.ascontiguousarray(W1[DFAC:])
    sel = np.ascontiguousarray(
        np.tile(np.eye(BLOC, dtype=np.float32), (16, 1)))       # (128, 8)

    # packed epilogue layout: row = 32*j + (k*8+b_local), col = t4*512 + c
    # with n = t4*2048 + j*512 + c
    def pack128(a32):  # (32, 8192) -> (128, 2048)
        return np.ascontiguousarray(
            a32.reshape(KB, NT4, NT4, TM).transpose(2, 0, 1, 3)
            .reshape(128, T4))

    catesT = cates.T                                            # (4, 8192)
    cates_rep = np.broadcast_to(
        catesT[:, None, :], (KFAC, BLOC, NITEMS)).reshape(KB, NITEMS)
    cates_p = pack128(cates_rep)

    in_maps = []
    for c in range(NCORES):
        bs = slice(c * BLOC, (c + 1) * BLOC)
        zt = np.ascontiguousarray(
            z[:, bs, :].transpose(2, 0, 1).reshape(DFAC, KB))   # (64, 32)
        delta_l = pack128(delta[:, bs, :].reshape(KB, NITEMS))
        x_l = np.ascontiguousarray(x[bs])
        in_maps.append({
            "w1a": w1a, "w1b": w1b, "w2": W2, "w3": W3,
            "b1": b1, "b2": b2, "b3b": b3b,
            "itemsT": itemsT, "zt": zt,
            "delta_l": delta_l, "cates_p": cates_p, "x_l": x_l,
            "sel": sel,
        })
    return in_maps


_NC_CACHE = None


def _get_program():
    global _NC_CACHE
    if _NC_CACHE is None:
        _NC_CACHE = build_program()
    return _NC_CACHE


def kernel(**inputs):
    from concourse.bass_utils import run_bass_kernel_spmd
    nc = _get_program()
    in_maps = make_in_maps(**inputs)
    res = run_bass_kernel_spmd(nc, in_maps, core_ids=list(range(NCORES)))
    outs = res.results
    all_item_prob = np.concatenate(
        [outs[c]["out_p"] for c in range(NCORES)], axis=0)
    reg_total = np.sum([outs[c]["rs"].sum(dtype=np.float64)
                        for c in range(NCORES)])
    reg_loss_aug = np.float32(reg_total / (BATCH * NITEMS) / (KFAC - 1))
    return all_item_prob, np.asarray(reg_loss_aug, dtype=np.float32)


# revision 37
# speedup vs baseline: 4.1130x; 1.0103x over previous
"""Trainium2 Bass kernel for nn_Augmenter_separate (dense_mlp).

Computation (reference):
    zc = einsum('kbd,dh->kbh', z, W1[:64])          # (4, 64, 128)
    ic = items @ W1[64:]                             # (8192, 128)
    for k in 4:
        h1 = relu(zc[k][:,None,:] + ic[None] + b1)   # (64, 8192, 128)
        h2 = relu(h1 @ W2 + b2)
        w  = ((h2 @ W3)[...,0] + b3) * cates[:,k]
        p  = sigmoid((log(delta[k]) - log1p(-delta[k]) + w) / 0.2)
        reg += sum(p)/p.size
        prob_k = where(x==1, 1-p, p)
    out = mean_k(prob_k), reg/3

Sharding: data-parallel over batch. Each of the 8 cores processes 8 of the
64 batch rows (full item axis). The scalar reg is summed on the host from
per-core partials; the (64, 8192) output is gathered by concatenation.

Per-core dataflow (all matmuls in float32r, fp32 storage):
  - icT [h=128, n=8192] once:  W1b^T @ itemsT          (PE)
  - zcb [h=128, kb=32]  once:  W1a^T @ zT + b1         (PE+ACT)
  - per (k,b) pair kb and 2048-wide n-chunk t4:
      h1 [128,2048] = relu(icT + zcb[:,kb])            (DVE tensor_scalar)
      psum = W2^T @ h1                                 (PE, 4x 512)
      h2 = relu(psum + b2)                             (ACT / DVE split)
      wt[32j+kb, :] += W3row-mask @ h2[:, j*512:...]   (PE, one-hot mask)
  - epilogue per t4 on [128,512] packed layout (row = 32j + kb):
      u = wt * cates + L0  ;  p = sigmoid(5u)  (reg via accum_out)
      psum_f[b,:] = sum_k p  (PE row-tiled selection matmul)
      out = xf + 0.25*(1-2x)*psum_f                    (DVE)
"""

import numpy as np

import concourse.bacc as bacc
import concourse.tile as tile
import concourse.mybir as mybir

F32 = mybir.dt.float32
F32R = mybir.dt.float32r
I32 = mybir.dt.int32
AF = mybir.ActivationFunctionType
OP = mybir.AluOpType

KFAC = 4
DFAC = 64
HDIM = 128
NITEMS = 8192
BATCH = 64
NCORES = 8
BLOC = BATCH // NCORES          # 8 local batch rows
KB = KFAC * BLOC                # 32 (k,b) pairs per core
NT4 = 4                         # n-chunks of 2048
T4 = NITEMS // NT4              # 2048
TM = 512                        # matmul moving-dim tile
TAU = 0.2

# (kb, half) pairs whose h2 evacuation runs on DVE instead of ACT,
# interleaved to keep both engines fed.
DVE_EVAC_H = frozenset({(4, 0), (4, 1), (10, 0), (10, 1), (16, 0), (16, 1),
                        (22, 0), (22, 1), (28, 0)})


def build_program():
    nc = bacc.Bacc("TRN2", target_bir_lowering=False, debug=False)

    # ---- DRAM I/O ----
    w1a_d = nc.dram_tensor("w1a", [DFAC, HDIM], F32R, kind="ExternalInput").ap()
    w1b_d = nc.dram_tensor("w1b", [DFAC, HDIM], F32R, kind="ExternalInput").ap()
    w2_d = nc.dram_tensor("w2", [HDIM, HDIM], F32R, kind="ExternalInput").ap()
    w3_d = nc.dram_tensor("w3", [HDIM, 1], F32, kind="ExternalInput").ap()
    b1_d = nc.dram_tensor("b1", [HDIM, 1], F32, kind="ExternalInput").ap()
    b2_d = nc.dram_tensor("b2", [HDIM, 1], F32, kind="ExternalInput").ap()
    b3_d = nc.dram_tensor("b3b", [HDIM, 1], F32, kind="ExternalInput").ap()
    itemsT_d = nc.dram_tensor("itemsT", [DFAC, NITEMS], F32R, kind="ExternalInput").ap()
    zt_d = nc.dram_tensor("zt", [DFAC, KB], F32R, kind="ExternalInput").ap()
    delta_d = nc.dram_tensor("delta_l", [128, T4], F32, kind="ExternalInput").ap()
    cates_d = nc.dram_tensor("cates_p", [128, T4], F32, kind="ExternalInput").ap()
    x_d = nc.dram_tensor("x_l", [BLOC, NITEMS], I32, kind="ExternalInput").ap()
    sel_d = nc.dram_tensor("sel", [128, BLOC], F32R, kind="ExternalInput").ap()

    out_d = nc.dram_tensor("out_p", [BLOC, NITEMS], F32, kind="ExternalOutput").ap()
    rs_d = nc.dram_tensor("rs", [128, NT4], F32, kind="ExternalOutput").ap()

    with tile.TileContext(nc) as tc:
        with tc.tile_pool(name="const", bufs=1) as const:
            # small constants
            w1a = const.tile([DFAC, HDIM], F32R)
            w1b = const.tile([DFAC, HDIM], F32R)
            w2r = const.tile([HDIM, HDIM], F32R)
            w3 = const.tile([HDIM, 1], F32)
            b1 = const.tile([HDIM, 1], F32)
            b2 = const.tile([HDIM, 1], F32)
            b3 = const.tile([HDIM, 1], F32)
            zt = const.tile([DFAC, KB], F32R)
            selr = const.tile([128, BLOC], F32R)
            for dst, src in (
                (w1a, w1a_d), (w1b, w1b_d), (w2r, w2_d), (w3, w3_d),
                (b1, b1_d), (b2, b2_d), (b3, b3_d), (zt, zt_d), (selr, sel_d),
            ):
                nc.sync.dma_start(out=dst, in_=src)

            # big persistent tensors
            zcb = const.tile([HDIM, KB], F32)
            l0p = const.tile([128, T4], F32)
            cates = const.tile([128, T4], F32)
            m1q = const.tile([BLOC, NITEMS], F32)
            xf = const.tile([BLOC, NITEMS], F32)
            # sliding-window one-hot mask: col 127 = W3, zeros elsewhere.
            # mwin[:, 127-m : 255-m] is a [128,128] matrix with col m = W3.
            mwin = const.tile([HDIM, 255], F32R)
            icTr = const.tile([HDIM, NITEMS], F32R)
            rs = const.tile([128, NT4], F32)

            # ---- setup ----
            from contextlib import ExitStack
            with ExitStack() as setup_ctx:
                spool = setup_ctx.enter_context(
                    tc.tile_pool(name="setup_sb", bufs=1))
                spsum = setup_ctx.enter_context(
                    tc.tile_pool(name="setup_ps", bufs=2, space="PSUM"))

                # icT = W1b^T @ itemsT
                for it in range(NITEMS // TM):
                    ps = spsum.tile([HDIM, TM], F32, tag="ps")
                    nc.tensor.matmul(
                        ps, w1b, itemsT[:, it * TM:(it + 1) * TM],
                        start=True, stop=True)
                    nc.scalar.activation(
                        out=icTr[:, it * TM:(it + 1) * TM], in_=ps,
                        func=AF.Copy)

                # zcb = W1a^T @ zT + b1
                psz = spsum.tile([HDIM, KB], F32, tag="ps")
                nc.tensor.matmul(psz, w1a, zt, start=True, stop=True)
                nc.scalar.activation(out=zcb, in_=psz, func=AF.Identity, bias=b1)

                # L0 = log(delta) - log(1-delta)  (+ b3*cates), packed layout
                delta = spool.tile([128, T4], F32, tag="delta")
                nc.sync.dma_start(out=delta, in_=delta_d)
                tln = spool.tile([128, T4], F32, tag="tln")
                nc.scalar.activation(out=tln, in_=delta, func=AF.Ln)
                tln2 = spool.tile([128, T4], F32, tag="tln2")
                nc.scalar.activation(
                    out=tln2, in_=delta, func=AF.Ln, scale=-1.0, bias=1.0)
                nc.vector.tensor_tensor(
                    out=l0p, in0=tln, in1=tln2, op=OP.subtract)

                nc.sync.dma_start(out=cates, in_=cates_d)
                tb3 = spool.tile([128, T4], F32, tag="tln")
                nc.vector.tensor_scalar(
                    out=tb3, in0=cates, scalar1=b3, scalar2=None, op0=OP.mult)
                nc.vector.tensor_tensor(out=l0p, in0=l0p, in1=tb3, op=OP.add)

                # x-derived tensors: m1q = 0.25*(1-2x), xf = float(x)
                xi = spool.tile([BLOC, NITEMS], I32, tag="xi")
                nc.sync.dma_start(out=xi, in_=x_d)
                nc.gpsimd.tensor_scalar(
                    out=m1q, in0=xi, scalar1=-0.5, scalar2=0.25,
                    op0=OP.mult, op1=OP.add)
                nc.gpsimd.tensor_copy(out=xf, in_=xi)

                # one-hot masks for the W3-row contraction
                # (memset can't produce f32r; zero via fp32 temp + copy)
                mz = spool.tile([HDIM, KB * KB], F32, tag="tln")
                nc.vector.memset(mz, 0.0)
                nc.vector.tensor_copy(
                    out=masks3.rearrange("p a b -> p (a b)"), in_=mz)
                for kb in range(KB):
                    nc.vector.tensor_copy(
                        out=masks3[:, kb, kb:kb + 1], in_=w3)

            # ---- main loop ----
            with (
                tc.tile_pool(name="h1p", bufs=4) as h1p,
                tc.tile_pool(name="h2p", bufs=4) as h2p,
                tc.tile_pool(name="hps", bufs=3, space="PSUM") as hps,
                tc.tile_pool(name="wps", bufs=2, space="PSUM") as wps,
                tc.tile_pool(name="epi", bufs=2) as epi,
                tc.tile_pool(name="outp", bufs=2) as outp,
            ):
                h1_cache = {}
                pending_epi = None

                def make_h1(t4, kb):
                    h1 = h1p.tile([HDIM, T4], F32R, tag="h1",
                                  name=f"h1_{t4}_{kb}")
                    nc.vector.tensor_scalar(
                        out=h1, in0=icTr[:, t4 * T4:(t4 + 1) * T4],
                        scalar1=zcb[:, kb:kb + 1], scalar2=0.0,
                        op0=OP.add, op1=OP.max)
                    return h1

                for t4 in range(NT4):
                    n0 = t4 * T4
                    wt = wps.tile([128, TM], F32, tag="w",
                                  name=f"wt_{t4}")
                    for kb in range(KB):
                        h1 = h1p.tile([HDIM, T4], F32R, tag="h1")
                        nc.vector.tensor_scalar(
                            out=h1, in0=icTr[:, n0:n0 + T4],
                            scalar1=zcb[:, kb:kb + 1], scalar2=0.0,
                            op0=OP.add, op1=OP.max)
                        h2h = [h2p.tile([HDIM, 2 * TM], F32R, tag="h2",
                                        name=f"h2_{t4}_{kb}_{hf}")
                               for hf in range(2)]
                        for half in range(2):
                            ph = hps.tile([HDIM, 2 * TM], F32, tag="ph")
                            for q in range(2):
                                c = half * 2 * TM + q * TM
                                nc.tensor.matmul(
                                    ph[:, q * TM:(q + 1) * TM], w2r,
                                    h1[:, c:c + TM], start=True, stop=True)
                            hs = h2h[half]
                            if (kb, half) not in DVE_EVAC_H:
                                nc.scalar.activation(
                                    out=hs, in_=ph, func=AF.Relu, bias=b2)
                            else:
                                nc.vector.tensor_scalar(
                                    out=hs, in0=ph, scalar1=b2, scalar2=0.0,
                                    op0=OP.add, op1=OP.max)
                            for q in range(2):
                                j = half * 2 + q
                                nc.tensor.matmul(
                                    wtl[j],
                                    masks3[:, kb, :],
                                    hs[:, q * TM:(q + 1) * TM],
                                    start=(kb == 0), stop=(kb == KB - 1))

                    # epilogue for this n-chunk
                    ub = epi.tile([128, TM], F32, tag="ub")
                    for j in range(NT4):
                        nc.vector.tensor_tensor(
                            out=ub[32 * j:32 * (j + 1), :], in0=wtl[j],
                            in1=cates[32 * j:32 * (j + 1),
                                      t4 * TM:(t4 + 1) * TM],
                            op=OP.mult)
                    nc.vector.tensor_tensor(
                        out=ub, in0=ub, in1=l0p[:, t4 * TM:(t4 + 1) * TM],
                        op=OP.add)
                    p = epi.tile([128, TM], F32R, tag="p")
                    nc.scalar.activation(
                        out=p, in_=ub, func=AF.Sigmoid, scale=1.0 / TAU,
                        accum_out=rs[:, t4:t4 + 1])
                    ost = outp.tile([BLOC, T4], F32, tag="ost")
                    for j in range(NT4):
                        pf = wps.tile([BLOC, TM], F32, tag="w")
                        nc.tensor.matmul(
                            pf, selr[32 * j:32 * (j + 1), :],
                            p[32 * j:32 * (j + 1), :],
                            start=True, stop=True, tile_position=(32 * j, 0))
                        js = slice(j * TM, (j + 1) * TM)
                        ns = slice(n0 + j * TM, n0 + (j + 1) * TM)
                        nc.vector.tensor_tensor(
                            out=ost[:, js], in0=pf, in1=m1q[:, ns],
                            op=OP.mult)
                        nc.vector.tensor_tensor(
                            out=ost[:, js], in0=ost[:, js], in1=xf[:, ns],
                            op=OP.add)
                    nc.sync.dma_start(out=out_d[:, n0:n0 + T4], in_=ost)
                nc.sync.dma_start(out=rs_d, in_=rs)

    nc.compile()
    return nc


def make_in_maps(cates, h, z, x, delta, items, W1, b1, W2, b2, W3, b3):
    """Host-side sharding + layout prep (pure slicing/transpose/broadcast)."""
    cates = np.asarray(cates, np.float32)
    z = np.asarray(z, np.float32)
    x = np.ascontiguousarray(np.asarray(x, np.int32))
    delta = np.asarray(delta, np.float32)
    items = np.asarray(items, np.float32)
    W1 = np.asarray(W1, np.float32)
    W2 = np.ascontiguousarray(np.asarray(W2, np.float32))
    W3 = np.ascontiguousarray(np.asarray(W3, np.float32).reshape(HDIM, 1))
    b1 = np.ascontiguousarray(np.asarray(b1, np.float32).reshape(HDIM, 1))
    b2 = np.ascontiguousarray(np.asarray(b2, np.float32).reshape(HDIM, 1))
    b3b = np.full((HDIM, 1), np.float32(np.asarray(b3).reshape(-1)[0]),
                  np.float32)

    itemsT = np.ascontiguousarray(items.T)                      # (64, 8192)
    w1a = np.ascontiguousarray(W1[:DFAC])
    w1b = np.ascontiguousarray(W1[DFAC:])
    sel = np.ascontiguousarray(
        np.tile(np.eye(BLOC, dtype=np.float32), (16, 1)))       # (128, 8)

    # packed epilogue layout: row = 32*j + (k*8+b_local), col = t4*512 + c
    # with n = t4*2048 + j*512 + c
    def pack128(a32):  # (32, 8192) -> (128, 2048)
        return np.ascontiguousarray(
            a32.reshape(KB, NT4, NT4, TM).transpose(2, 0, 1, 3)
            .reshape(128, T4))

    catesT = cates.T                                            # (4, 8192)
    cates_rep = np.broadcast_to(
        catesT[:, None, :], (KFAC, BLOC, NITEMS)).reshape(KB, NITEMS)
    cates_p = pack128(cates_rep)

    in_maps = []
    for c in range(NCORES):
        bs = slice(c * BLOC, (c + 1) * BLOC)
        zt = np.ascontiguousarray(
            z[:, bs, :].transpose(2, 0, 1).reshape(DFAC, KB))   # (64, 32)
        delta_l = pack128(delta[:, bs, :].reshape(KB, NITEMS))
        x_l = np.ascontiguousarray(x[bs])
        in_maps.append({
            "w1a": w1a, "w1b": w1b, "w2": W2, "w3": W3,
            "b1": b1, "b2": b2, "b3b": b3b,
            "itemsT": itemsT, "zt": zt,
            "delta_l": delta_l, "cates_p": cates_p, "x_l": x_l,
            "sel": sel,
        })
    return in_maps


_NC_CACHE = None


def _get_program():
    global _NC_CACHE
    if _NC_CACHE is None:
        _NC_CACHE = build_program()
    return _NC_CACHE


def kernel(**inputs):
    from concourse.bass_utils import run_bass_kernel_spmd
    nc = _get_program()
    in_maps = make_in_maps(**inputs)
    res = run_bass_kernel_spmd(nc, in_maps, core_ids=list(range(NCORES)))
    outs = res.results
    all_item_prob = np.concatenate(
        [outs[c]["out_p"] for c in range(NCORES)], axis=0)
    reg_total = np.sum([outs[c]["rs"].sum(dtype=np.float64)
                        for c in range(NCORES)])
    reg_loss_aug = np.float32(reg_total / (BATCH * NITEMS) / (KFAC - 1))
    return all_item_prob, np.asarray(reg_loss_aug, dtype=np.float32)
